# revision 12
# baseline (speedup 1.0000x reference)
"""Trainium2 Bass kernel for nn_BallModel: 10M-step ballistic trajectory.

The reference recurrence (pos += vel*dt; vel += g*dt, recording pos) has the
closed form
    pos_i = pos0 + i*dt*vel0 + g*dt^2 * i*(i-1)/2  =  A + B*i + C*i^2
with A = pos0, B = dt*vel0 - C, C = (g*dt)*dt/2 (per component; C_x = 0).

Output is [10_000_000, 2] f32 (~80 MB), interleaved x,y.  Each of the 8 cores
produces a contiguous 2.5M-element slice (10 MB) -> memory-bound at the
per-core HBM write bandwidth (~358 GB/s line rate => ~28 us drain floor).

v4 = v1's chunk-interleaved, HBM-contiguous layout + HAND-ROLLED
synchronization on raw Bacc (no TileContext).  Trace findings driving this:
 - v1 (TileContext): drain runs at line rate, but the framework epilogue
   (per-semaphore reset ceremony + all-engine barriers) burns ~9 us after
   the last byte and the preamble event chains delay the input DMA.
 - v3 (partition-contiguous layout): raw sync fixed the epilogue, but
   column-slice DMA destinations (128 x 8KB segments at 78 KB stride) drain
   ~10% below line rate; fully-contiguous chunk destinations don't.

Layout (v1's): core element e = c*65536 + p*512 + ce over 39 chunks of
[128 partitions x 512 cols] (one PSUM bank each); chunk c's output region
out[c*128:(c+1)*128, :] is a contiguous 256 KB of HBM.  Pair index
i = q + jb with q(c,p) = core*1.25e6 + c*32768 + p*256, jb = ce>>1, and
ce&1 alternating x/y, so one shared rhs table [K=10, 512] drives every
chunk; per-(chunk,partition) stationary lhsT tables carry q.  Values wider
than bf16's 8 mantissa bits are split into 2-3 bf16 rows whose products
accumulate exactly in the fp32 PSUM accumulator (result ~1e-7 rel of the
f64 closed form).

Pipeline: chunks in groups of [1,1,2,4,4,...,4,3] (ramped so the first
output DMA launches after a single matmul + small copy).  Group g: matmuls
-> PSUM pool g%2 ([128,2048] f32, 4 banks) -> one copy into the [128,19968]
SBUF staging tensor (even groups on vector, odd on scalar, so every
consumer waits on a single semaphore that Bacc fuses into the consuming
instruction) -> one output DMA (multi-chunk groups use the p<->j rearrange
so the destination stays one contiguous HBM run; measured line rate in v1).
The partial last chunk (19 useful rows) ships as a separate tiny final DMA
so the drain ends on a fast completion.  5 semaphores total; the epilogue
is one sync-engine wait plus a 5-sem clear for NEFF re-execution safety.
All DMAs ride the sync HWDGE queue (gpsimd SWDGE stalls; scalar HWDGE
hard-hangs the device — v1 finding).
"""

import sys
import types

import ml_dtypes
import numpy as np

import concourse.bacc as bacc
import concourse.bass as bass
import concourse.mybir as mybir
from concourse.bass_utils import run_bass_kernel_spmd

# ---- problem constants (hardcoded; kernel.py must be self-contained) ----
N_PAIRS = 10_000_000
ELEMS = 2 * N_PAIRS  # 20,000,000 interleaved f32 values
N_CORES = 8
CE = ELEMS // N_CORES  # 2,500,000 elements per core
P = 128  # partitions
COLS = 512  # one PSUM bank of f32
CHUNK = P * COLS  # 65,536 elements per chunk
NB = 39  # chunks per core (38 full + one partial)
LAST_ROWS = -(-(CE - 38 * CHUNK) // COLS)  # 19 useful rows of final chunk
K = 10  # matmul contraction rows
# group schedule over chunks, by chunk index.  The tiny partial chunk (38)
# ships FIRST (its copy is 19 rows, its DMA 38 KB) so the drain never ends
# on a slow straggler; then 8 single-chunk ramp groups (first output DMA
# ~1.3 us after the first matmul); then 2-chunk groups whose ~1.1 us copy
# latency keeps supply (~500 B/ns) above the ~358 B/ns drain with no
# transition stall (4-chunk copies at 2.3 us caused one).
GROUPS = [[38]] + [[c] for c in range(8)] + [[c, c + 1] for c in range(8, 38, 2)]
RAMP_CHUNKS = [38] + list(range(8))  # chunks whose lhsT rides the head DMA
HEAD_BLOCKS = len(RAMP_CHUNKS)  # 9

# fp32-rounded constants, matching the reference's fp32 parameter rounding
DT = float(np.float32(0.01))
GDT_Y = float(np.float32(np.float32(-9.81) * np.float32(0.01)))  # fp32(g_y*dt)
C_Y = GDT_Y * DT / 2.0  # i^2 coefficient for y

_bf16 = ml_dtypes.bfloat16

# exposed for test.py introspection (exec_time_ns etc.)
LAST_RESULTS = None


def _ensure_axon_hooks_stub():
    """bass_utils imports antenv.axon_hooks when BASS_TRACE is set; some
    images lack that module.  Register a stub that degrades to the untraced
    path instead of crashing (test.py replaces it with a real NTFF hook)."""
    try:
        import antenv.axon_hooks  # noqa: F401

        return
    except ImportError:
        pass
    try:
        import antenv  # noqa: F401
    except ImportError:
        return
    stub = types.ModuleType("antenv.axon_hooks")
    stub.get_axon_ntff_profile_hook = lambda: None
    stub.set_axon_ntff_profile_hook = lambda h: None
    sys.modules["antenv.axon_hooks"] = stub


def _build_program() -> bass.Bass:
    # Bacc (not raw Bass): its finalize pipeline runs the sync-wait
    # legalization (fusing our standalone wait_ge's into their consumers)
    # and the register allocation walrus requires.
    nc = bacc.Bacc("TRN2", target_bir_lowering=False)
    hd = nc.declare_dram_parameter(
        "hd", [K, COLS + HEAD_BLOCKS * P], mybir.dt.bfloat16, isOutput=False
    )
    lt_t = nc.declare_dram_parameter(
        "lt_t", [K, (NB - HEAD_BLOCKS) * P], mybir.dt.bfloat16, isOutput=False
    )
    out = nc.declare_dram_parameter(
        "out", [NB * P, COLS], mybir.dt.float32, isOutput=True
    )

    hd_s = nc.alloc_sbuf_tensor(
        "hd_s", [K, COLS + HEAD_BLOCKS * P], mybir.dt.bfloat16
    )
    ltt_s = nc.alloc_sbuf_tensor(
        "ltt_s", [K, (NB - HEAD_BLOCKS) * P], mybir.dt.bfloat16
    )
    # one dedicated staging tile per group: v1/v4 A/B showed the descriptor
    # generator emits contiguous-2KB-per-destination descriptors (line-rate
    # drain) for standalone tiles, but 8KB strided-destination descriptors
    # (~10% slower) when the source is a column slice of one big tensor
    ot_s = [
        nc.alloc_sbuf_tensor(f"ot{g}", [P, len(ch) * COLS], mybir.dt.float32)
        for g, ch in enumerate(GROUPS)
    ]
    pools = [
        nc.alloc_psum_tensor("pa", [P, 4 * COLS], mybir.dt.float32),
        nc.alloc_psum_tensor("pb", [P, 4 * COLS], mybir.dt.float32),
    ]

    s_hd = nc.alloc_semaphore("s_hd")  # head input DMA completion (+16)
    s_lt = nc.alloc_semaphore("s_lt")  # lhsT tail input DMA completion (+16)
    s_pe = nc.alloc_semaphore("s_pe")  # matmul retirements (+1 each)
    s_vc = nc.alloc_semaphore("s_vc")  # vector copy retirements
    s_sc = nc.alloc_semaphore("s_sc")  # scalar copy retirements
    s_do = nc.alloc_semaphore("s_do")  # output DMA completions (+16 each)
    sems = [s_hd, s_lt, s_pe, s_vc, s_sc, s_do]

    # each input DMA gets its OWN completion semaphore: the 16 per-SDMA-engine
    # increments of two DMAs sharing one semaphore can interleave, so a
    # shared counter can hit 16 before the first DMA's data has fully landed
    nc.sync.dma_start(hd_s[:], hd[:]).then_inc(s_hd, 16)
    nc.sync.dma_start(ltt_s[:], lt_t[:]).then_inc(s_lt, 16)
    rh_s = hd_s[:, :COLS]

    order = RAMP_CHUNKS + [c for c in range(NB) if c not in RAMP_CHUNKS]
    pos_of = {c: i for i, c in enumerate(order)}

    def lhsT(c):
        i = pos_of[c]
        if i < HEAD_BLOCKS:
            return hd_s[:, COLS + i * P : COLS + (i + 1) * P]
        i -= HEAD_BLOCKS
        return ltt_s[:, i * P : (i + 1) * P]

    n_groups = len(GROUPS)
    copy_sem = lambda g: s_vc if g % 2 == 0 else s_sc
    copy_val = [0] * n_groups
    cnt = {0: 0, 1: 0}
    for g in range(n_groups):
        cnt[g % 2] += 1
        copy_val[g] = cnt[g % 2]

    # per-pool PSUM bank rotation + last-user tracking for WAR waits (every
    # user of a pool has the same parity, so each WAR is a single-sem wait)
    bank_last = [[None] * 4, [None] * 4]
    rot = [0, 0]
    cum_mm = 0
    n_dma = 0
    used_ltt = False
    for g, chunks in enumerate(GROUPS):
        nbl = len(chunks)
        pool = g % 2
        if nbl == 1:
            b0b = rot[pool] % 4
        else:
            b0b = (rot[pool] % 2) * 2
        rot[pool] += 1
        pt = pools[pool][:, b0b * COLS : (b0b + nbl) * COLS]
        # PE gating (waits fuse into the next PE instruction; at most one
        # group carries two waits, costing a single event-sem instruction)
        if g == 0:
            nc.tensor.wait_ge(s_hd, 16)  # rh + head lhsT resident
        if not used_ltt and any(pos_of[c] >= HEAD_BLOCKS for c in chunks):
            nc.tensor.wait_ge(s_lt, 16)  # lhsT tail resident
            used_ltt = True
        war = [
            bank_last[pool][b]
            for b in range(b0b, b0b + nbl)
            if bank_last[pool][b] is not None
        ]
        if war:
            w = max(war)
            nc.tensor.wait_ge(copy_sem(w), copy_val[w])
        for b in range(b0b, b0b + nbl):
            bank_last[pool][b] = g
        rows = LAST_ROWS if chunks == [NB - 1] else P
        for i, c in enumerate(chunks):
            nc.tensor.matmul(
                pt[:, i * COLS : (i + 1) * COLS],
                lhsT(c),
                rh_s,
                start=True,
                stop=True,
            ).then_inc(s_pe, 1)
        cum_mm += nbl
        ncols = nbl * COLS
        ot = ot_s[g]
        eng = nc.vector if g % 2 == 0 else nc.scalar
        eng.wait_ge(s_pe, cum_mm)
        if g % 2 == 0:
            ci = nc.vector.tensor_copy(ot[:rows, :], pt[:rows, :])
        else:
            ci = nc.scalar.copy(ot[:rows, :], pt[:rows, :])
        ci.then_inc(copy_sem(g), 1)
        # output DMA; multi-chunk groups rearrange so the HBM destination
        # stays one contiguous run (line-rate descriptors, v1-measured)
        nc.sync.wait_ge(copy_sem(g), copy_val[g])
        c0 = chunks[0]
        if nbl == 1:
            dst = out[c0 * P : c0 * P + rows, :]
            src = ot[:rows, :]
        else:
            dst = out[c0 * P : (c0 + nbl) * P, :].rearrange(
                "(j p) q -> p j q", p=P
            )
            src = ot[:, :ncols].rearrange("p (j q) -> p j q", q=COLS)
        nc.sync.dma_start(dst, src).then_inc(s_do, 16)
        n_dma += 1

    # Epilogue: kernel completion = all output bytes landed.  gpsimd then
    # re-zeroes our semaphores so a re-execution of this NEFF starts clean
    # (the framework contract is sems == 0 at kernel entry).
    nc.sync.wait_ge(s_do, 16 * n_dma)
    nc.gpsimd.wait_ge(s_do, 16 * n_dma)
    nums = sorted(s.num for s in sems)
    if nums == list(range(nums[0], nums[0] + len(nums))):
        nc.gpsimd.sem_clear(range(nums[0], nums[-1] + 1))
    else:
        for s in sems:
            nc.gpsimd.sem_clear(s)
    nc.finalize()  # runs Bacc.compile(): reg alloc + sync-wait legalization
    return nc


def _split_bf16(x: np.ndarray, n: int):
    """Split x into n bf16 parts summing (nearly) exactly to x."""
    parts = []
    rem = np.asarray(x, dtype=np.float64).copy()
    for _ in range(n):
        p = rem.astype(_bf16)
        parts.append(p)
        rem = rem - p.astype(np.float64)
    return parts


def _host_tables(pos0: np.ndarray, vel0: np.ndarray):
    """Build per-core input tables (float64 math, cast at the end)."""
    ax, ay = float(pos0[0]), float(pos0[1])
    bx_c = DT * float(vel0[0])  # B_x (C_x = 0)
    by_c = DT * float(vel0[1]) - C_Y  # B_y

    # fixed rhs column patterns (jb < 256 within every 512-col chunk)
    ce = np.arange(COLS)
    j = (ce >> 1).astype(np.float64)
    odd = (ce & 1).astype(np.float64)
    even = 1.0 - odd
    jodd = (j * odd).astype(_bf16)  # exact: j < 256
    resid = np.where(ce & 1 == 1, C_Y * j * j, bx_c * j)
    resid_hi, resid_lo = _split_bf16(resid, 2)
    rh_np = np.stack(
        [
            jodd,
            jodd,
            resid_hi,
            resid_lo,
            odd.astype(_bf16),
            odd.astype(_bf16),
            odd.astype(_bf16),
            even.astype(_bf16),
            even.astype(_bf16),
            even.astype(_bf16),
        ]
    )  # [K, COLS]

    in_maps = []
    c_idx = np.arange(NB, dtype=np.float64)[:, None]  # [NB, 1]
    p_idx = np.arange(P, dtype=np.float64)[None, :]  # [1, P]
    for k in range(N_CORES):
        q = k * (CE // 2) + c_idx * (CHUNK // 2) + p_idx * (COLS // 2)  # [NB, P]
        s1_hi, s1_lo = _split_bf16(by_c + 2.0 * C_Y * q, 2)
        ones = np.ones_like(s1_hi)
        by3 = _split_bf16(ay + by_c * q + C_Y * q * q, 3)
        bx3 = _split_bf16(ax + bx_c * q, 3)
        rows = [s1_hi, s1_lo, ones, ones] + by3 + bx3
        lt_np = np.stack([r.reshape(-1) for r in rows])  # [K, NB*P], chunk-major
        order = RAMP_CHUNKS + [c for c in range(NB) if c not in RAMP_CHUNKS]
        lt_np = np.concatenate(
            [lt_np[:, c * P : (c + 1) * P] for c in order], axis=1
        )
        in_maps.append(
            {
                "hd": np.ascontiguousarray(
                    np.concatenate([rh_np, lt_np[:, : HEAD_BLOCKS * P]], axis=1)
                ),
                "lt_t": np.ascontiguousarray(lt_np[:, HEAD_BLOCKS * P :]),
            }
        )
    return in_maps


def kernel(ball_mass, ball_initial_position, ball_initial_velocity) -> np.ndarray:
    global LAST_RESULTS
    pos0 = np.asarray(ball_initial_position, dtype=np.float32)
    vel0 = np.asarray(ball_initial_velocity, dtype=np.float32)

    _ensure_axon_hooks_stub()
    nc = _build_program()
    in_maps = _host_tables(pos0, vel0)
    res = run_bass_kernel_spmd(nc, in_maps, core_ids=list(range(N_CORES)))
    LAST_RESULTS = res

    parts = [
        np.asarray(r["out"], dtype=np.float32).reshape(-1)[:CE] for r in res.results
    ]
    return np.concatenate(parts).reshape(N_PAIRS, 2)


if __name__ == "__main__":
    import os

    pos0 = (
        np.load("/tmp/pos0.npy")
        if os.path.exists("/tmp/pos0.npy")
        else np.array([-1.866805, -0.25733662], np.float32)
    )
    vel0 = (
        np.load("/tmp/vel0.npy")
        if os.path.exists("/tmp/vel0.npy")
        else np.array([-0.847358, -1.5444987], np.float32)
    )
    outv = kernel(np.ones(()), pos0, vel0)
    i = np.arange(N_PAIRS, dtype=np.float64)[:, None]
    closed = (
        pos0.astype(np.float64)
        + i * DT * vel0.astype(np.float64)
        + np.array([0.0, GDT_Y * DT]) * i * (i - 1) / 2.0
    )
    err = np.abs(outv - closed)
    denom = np.maximum(np.abs(closed), 1e-12)
    print("closed-form maxabs-ratio rel err:", err.max() / np.abs(closed).max())
    print("closed-form max elementwise rel err:", (err / denom).max())
